# revision 1
# baseline (speedup 1.0000x reference)
"""Two-layer GCN block (PyG GCNConv x2, no nonlinearity) on 8 trn2 NeuronCores.

Math: out1 = D^-1/2 (A+I) D^-1/2 (x W1) + b1 ; out2 = same on out1 with W2, b2.
Factorization used on device:
    u  = dis (.) x                     (row scale, dis = deg^-1/2)
    A[d] = sum_{e: src->d} u[src]      (plain segment sum incl. self loops)
    out1 = dis (.) (A @ W1) + b1
    v    = dis (.) out1                (gather source for layer 2)
    out2 = dis (.) (A2 @ W2) + b2
Sharding: destinations split across 8 cores (6250 each, padded to 6272 = 49*128).
Each core gathers u rows by source id from a replicated (AllGather'd) table,
accumulates per 128-dest tile in PSUM via identity matmuls, applies the dense
64x64 weight per tile, and scatters rows back. Host does index prep only.
"""
import sys
import numpy as np

sys.path.insert(0, '/root/.axon_site')
sys.path.insert(0, '/opt/trn_rl_repo')

N = 50000
E = 800000
D = 64
C = 8              # cores
NSH = 6250         # real dests per core
NT = 49            # dest tiles per core
NSHP = NT * 128    # 6272 padded dests per core
NFULL = C * NSHP   # 50176
P = 128
GZROW = 6250       # padded-global id of an all-zero row (shard 0 pad row)

_compiled = None   # (nc, S, D_list) cache across calls


def _gpad(src):
    """original node id -> padded-global row id"""
    return (src // NSH) * NSHP + (src % NSH)


def _build_schedule(col_sorted_by_core, srcs_by_core):
    """Common per-tile slot counts across cores + per-core index arrays."""
    # degrees per core (local dest id -> degree incl self loop)
    degs = []
    perms = []
    for k in range(C):
        deg = np.bincount(col_sorted_by_core[k], minlength=NSH) + 1
        perm = np.argsort(-deg, kind='stable')
        degs.append(deg)
        perms.append(perm)
    D_list = []
    for t in range(NT):
        lo, hi = t * 128, min((t + 1) * 128, NSH)
        dmax = 0
        for k in range(C):
            dmax = max(dmax, int(degs[k][perms[k][lo:hi]].max()) - 1)
        D_list.append(dmax)
    return degs, perms, D_list


def kernel(x, edge_index, W1, b1, W2, b2):
    import concourse.bass as bass
    import concourse.bacc as bacc
    import concourse.mybir as mybir
    from concourse import tile
    from concourse.bass_utils import run_bass_kernel_spmd

    x = np.asarray(x, dtype=np.float32)
    edge_index = np.asarray(edge_index)
    W1 = np.asarray(W1, dtype=np.float32)
    W2 = np.asarray(W2, dtype=np.float32)
    b1 = np.asarray(b1, dtype=np.float32)
    b2 = np.asarray(b2, dtype=np.float32)

    row = edge_index[0].astype(np.int64)   # sources
    col = edge_index[1].astype(np.int64)   # destinations

    deg = np.bincount(col, minlength=N).astype(np.float32) + 1.0  # + self loop
    dis = (1.0 / np.sqrt(deg)).astype(np.float32)

    # ---- per-core edge lists (dest-sharded) ----
    core_of = col // NSH
    order = np.argsort(col, kind='stable')
    col_s, row_s = col[order], row[order]
    # boundaries between cores in the dest-sorted edge list
    bounds = np.searchsorted(col_s, np.arange(0, N + 1, NSH))
    col_by_core, src_by_core = [], []
    for k in range(C):
        sl = slice(bounds[k], bounds[k + 1])
        col_by_core.append((col_s[sl] - k * NSH).astype(np.int64))
        src_by_core.append(row_s[sl])

    degs, perms, D_list = _build_schedule(col_by_core, src_by_core)
    S = int(np.sum(D_list))
    offs = np.concatenate([[0], np.cumsum(D_list)]).astype(np.int64)

    # ---- per-core host arrays ----
    idx_arr = np.full((C, P, S), GZROW, dtype=np.int32)
    idx2_arr = np.full((C, P, S), 47466, dtype=np.int32)
    disrc_arr = np.zeros((C, P, S), dtype=np.float32)
    disd_arr = np.zeros((C, P, NT), dtype=np.float32)
    disd2_arr = np.zeros((C, P, NT), dtype=np.float32)
    disd_rows_arr = np.zeros((C, 1, NSHP), dtype=np.float32)
    # replicated padded x table (zero pad rows) in padded-global coords
    x_pad = np.zeros((NFULL, D), dtype=np.float32)
    for k in range(C):
        x_pad[k * NSHP:k * NSHP + NSH] = x[k * NSH:(k + 1) * NSH]
    # own-shard x rows in degree-sorted order (self-loop block per tile)
    x_own_arr = np.zeros((C, NSHP, D), dtype=np.float32)

    # perm-inverse per core: orig local dest id -> sorted row position.
    # Output tiles write contiguously in sorted order (plain DMA, not
    # indirect scatter); layer-2 gather indices absorb the permutation.
    pinvs = []
    for k in range(C):
        pinv = np.empty(NSH, dtype=np.int64)
        pinv[perms[k]] = np.arange(NSH)
        pinvs.append(pinv)

    # AllGather pieces (tile-aligned): layer-1 output is gathered in 4
    # chunks so the first 3 overlap the gather stream. vfull layout is
    # piece-major then rank-major.
    TB = [0, 18, 33, 46, 49]                    # tile boundaries (front-loaded
    # so the last AllGather piece is small and barely exposed)
    piece_lo = np.array([0, 2304, 4224, 5888])
    piece_rows = np.array([2304, 1920, 1664, 384])
    region_base = np.array([0, 18432, 33792, 47104])
    pin_all = np.concatenate(pinvs)             # orig id -> permuted position
    korig = np.arange(N) // NSH
    pc = np.digitize(pin_all, piece_lo[1:])     # piece index of each position
    g2_of = (region_base[pc] + korig * piece_rows[pc]
             + (pin_all - piece_lo[pc]))        # orig id -> vfull row

    for k in range(C):
        lc, ls = col_by_core[k], src_by_core[k]
        # CSR by local dest: edges sorted by local dest already (stable sort)
        starts = np.searchsorted(lc, np.arange(NSH + 1))
        perm = perms[k]
        dval = dis[k * NSH + perm]                      # dis of sorted dests
        for t in range(NT):
            lo = t * 128
            nreal = min(128, NSH - lo)
            pv = perm[lo:lo + nreal]                    # local dest ids, sorted pos
            disd_arr[k, :nreal, t] = dval[lo:lo + nreal]
            # slots
            o = offs[t]
            for j, v in enumerate(pv):
                e0, e1 = starts[v], starts[v + 1]
                nsrc = e1 - e0
                idx_arr[k, j, o:o + nsrc] = _gpad(ls[e0:e1])
                idx2_arr[k, j, o:o + nsrc] = g2_of[ls[e0:e1]]
                disrc_arr[k, j, o:o + nsrc] = dis[ls[e0:e1]]
        disd2_arr[k] = disd_arr[k] ** 2
        x_own_arr[k, :NSH] = x[k * NSH + perm]
        # disd by sorted row position (for the bias outer product, layer 1)
        dr = np.zeros(NSHP, dtype=np.float32)
        dr[:NSH] = dis[k * NSH + perm]
        disd_rows_arr[k, 0, :] = dr

    # sanity: each dest's slot count fits (degree incl self loop <= D_t)
    # (guaranteed by schedule construction)

    # ---- build device program ----
    nc = bacc.Bacc(None, target_bir_lowering=False)
    dt = mybir.dt
    xtab = nc.declare_dram_parameter("xtab", [NFULL, D], dt.float32, isOutput=False)
    identp = nc.declare_dram_parameter("identp", [P, P], dt.float32, isOutput=False)
    xown = nc.declare_dram_parameter("xown", [NSHP, D], dt.float32, isOutput=False)
    dsrc = nc.declare_dram_parameter("dsrc", [P, S], dt.float32, isOutput=False)
    idxp = nc.declare_dram_parameter("idxp", [P, S], dt.int32, isOutput=False)
    idxp2 = nc.declare_dram_parameter("idxp2", [P, S], dt.int32, isOutput=False)
    disd = nc.declare_dram_parameter("disd", [P, NT], dt.float32, isOutput=False)
    disd2 = nc.declare_dram_parameter("disd2", [P, NT], dt.float32, isOutput=False)
    disdr = nc.declare_dram_parameter("disdr", [1, NSHP], dt.float32, isOutput=False)
    w1p = nc.declare_dram_parameter("w1p", [D, D], dt.float32, isOutput=False)
    w2p = nc.declare_dram_parameter("w2p", [D, D], dt.float32, isOutput=False)
    b1p = nc.declare_dram_parameter("b1p", [1, D], dt.float32, isOutput=False)
    b2p = nc.declare_dram_parameter("b2p", [1, D], dt.float32, isOutput=False)
    out_sh = nc.declare_dram_parameter("out_sh", [NSHP, D], dt.float32, isOutput=True)

    v_shp = [nc.dram_tensor(f"v_sh{i}", [int(piece_rows[i]), D], dt.float32)
             for i in range(4)]
    vfull = nc.dram_tensor("vfull", [NFULL, D], dt.float32, addr_space="Shared")

    rg = [list(range(C))]

    with tile.TileContext(nc) as tc:
        with tc.tile_pool(name="const", bufs=1) as cp, \
             tc.tile_pool(name="sb", bufs=4) as pool, \
             tc.tile_pool(name="gp", bufs=3) as gpool, \
             tc.tile_pool(name="ep", bufs=3) as ep, \
             tc.tile_pool(name="psA", bufs=2, space="PSUM") as psA, \
             tc.tile_pool(name="psB", bufs=2, space="PSUM") as psB, \
             tc.tile_pool(name="psC", bufs=2, space="PSUM") as psC:

            ident = cp.tile([P, P], dt.float32)
            nc.sync.dma_start(out=ident[:], in_=identp[:, :])
            w1t = cp.tile([D, D], dt.float32)
            nc.sync.dma_start(out=w1t[:], in_=w1p[:, :])
            w2t = cp.tile([D, D], dt.float32)
            nc.sync.dma_start(out=w2t[:], in_=w2p[:, :])
            b1t = cp.tile([1, D], dt.float32)
            nc.sync.dma_start(out=b1t[:], in_=b1p[:, :])
            b2t = cp.tile([1, D], dt.float32)
            nc.sync.dma_start(out=b2t[:], in_=b2p[:, :])
            it = cp.tile([P, S], dt.int32)
            h = int(offs[1])
            nc.sync.dma_start(out=it[:, :h], in_=idxp[:, :h])
            nc.sync.dma_start(out=it[:, h:], in_=idxp[:, h:])
            it2 = cp.tile([P, S], dt.int32)
            nc.sync.dma_start(out=it2[:], in_=idxp2[:, :])
            dst_ = cp.tile([P, S], dt.float32)
            nc.sync.dma_start(out=dst_[:], in_=dsrc[:, :])
            dd1 = cp.tile([P, NT], dt.float32)
            nc.sync.dma_start(out=dd1[:], in_=disd[:, :])
            dd2 = cp.tile([P, NT], dt.float32)
            nc.sync.dma_start(out=dd2[:], in_=disd2[:, :])
            ddr = cp.tile([1, NSHP], dt.float32)
            nc.sync.dma_start(out=ddr[:], in_=disdr[:, :])
            ones_row = cp.tile([1, P], dt.float32)
            nc.vector.memset(ones_row[:], 1.0)

            def layer(src_full, idx_tile, wtile, btile, bias_rhs, scale_tile,
                      dest_of, self_of, slot_scale=None, post_tile=None):
                maxD = max(D_list)
                for t in range(NT):
                    acc = psA.tile([P, D], dt.float32)
                    dcount = D_list[t]
                    # one buffer per dest-tile: slice writes share a tile
                    # generation, so only the first DMA needs a slot wait
                    gbuf = gpool.tile([P, maxD * D], dt.float32, tag="g")
                    for s in range(dcount):
                        nc.gpsimd.indirect_dma_start(
                            out=gbuf[:, s * D:(s + 1) * D], out_offset=None,
                            in_=src_full[:],
                            in_offset=bass.IndirectOffsetOnAxis(
                                ap=idx_tile[:, int(offs[t]) + s:
                                            int(offs[t]) + s + 1],
                                axis=0))
                    if slot_scale is not None:
                        gsb = gpool.tile([P, maxD * D], dt.float32, tag="gs")
                        for s in range(dcount):
                            nc.scalar.activation(
                                out=gsb[:, s * D:(s + 1) * D],
                                in_=gbuf[:, s * D:(s + 1) * D],
                                func=mybir.ActivationFunctionType.Copy,
                                scale=slot_scale[:, int(offs[t]) + s:
                                                 int(offs[t]) + s + 1])
                        rsrc = gsb
                    else:
                        rsrc = gbuf
                    # self-loop block: contiguous rows, loaded off-queue
                    stens, soff = self_of(t)
                    st = ep.tile([P, D], dt.float32, tag="sl")
                    nc.sync.dma_start(out=st[:], in_=stens[soff:soff + P, :])
                    if slot_scale is not None:
                        st2 = ep.tile([P, D], dt.float32, tag="sl2")
                        nc.scalar.activation(
                            out=st2[:], in_=st[:],
                            func=mybir.ActivationFunctionType.Copy,
                            scale=scale_self[:, t:t + 1])
                        st = st2
                    nc.tensor.matmul(acc[:], lhsT=ident[:], rhs=st[:],
                                     start=True, stop=(dcount == 0))
                    for s in range(dcount):
                        nc.tensor.matmul(acc[:], lhsT=ident[:],
                                         rhs=rsrc[:, s * D:(s + 1) * D],
                                         start=False, stop=(s == dcount - 1))
                    # scale rows (dests on partitions)
                    csb = ep.tile([P, D], dt.float32, tag="c")
                    nc.scalar.activation(
                        out=csb[:], in_=acc[:],
                        func=mybir.ActivationFunctionType.Copy,
                        scale=scale_tile[:, t:t + 1])
                    # transpose -> [64, 128]
                    tr1 = psB.tile([D, P], dt.float32)
                    nc.tensor.transpose(tr1[:], csb[:], ident[:])
                    ct = ep.tile([D, P], dt.float32, tag="ct")
                    nc.vector.tensor_copy(out=ct[:], in_=tr1[:])
                    # W^T @ C^T (+ bias outer)
                    pv = psC.tile([D, P], dt.float32)
                    nc.tensor.matmul(pv[:], lhsT=wtile[:], rhs=ct[:],
                                     start=True, stop=False)
                    nc.tensor.matmul(pv[:], lhsT=btile[:], rhs=bias_rhs(t),
                                     start=False, stop=True)
                    vt = ep.tile([D, P], dt.float32, tag="vt")
                    nc.vector.tensor_copy(out=vt[:], in_=pv[:])
                    # transpose back -> [128, 64]
                    tr2 = psB.tile([P, D], dt.float32)
                    nc.tensor.matmul(tr2[:], lhsT=vt[:], rhs=ident[:D, :D],
                                     is_transpose=True)
                    vsb = ep.tile([P, D], dt.float32, tag="vs")
                    nc.vector.tensor_copy(out=vsb[:], in_=tr2[:])
                    # rows are in degree-sorted order -> contiguous write
                    # (HWDGE; keeps the GpSimd queue free for gathers)
                    dtens, doff = dest_of(t)
                    nc.sync.dma_start(out=dtens[doff:doff + P, :], in_=vsb[:])
                    if post_tile is not None:
                        post_tile(t)

            # layer 1 -> v_sh (= dis (.) out1, degree-sorted row order);
            # gathers raw x, scales by dis[src] per slot on the Scalar engine
            def v_dest(t):
                i = 0
                while t >= TB[i + 1]:
                    i += 1
                return v_shp[i], (t - TB[i]) * P

            def fire_ag(t):
                for i in range(4):
                    if t == TB[i + 1] - 1:
                        lo = int(region_base[i])
                        hi = lo + 8 * int(piece_rows[i])
                        nc.gpsimd.collective_compute(
                            "AllGather", mybir.AluOpType.bypass,
                            replica_groups=rg,
                            ins=[v_shp[i][:]], outs=[vfull[lo:hi, :]])

            scale_self = dd1   # dis[dest] for the layer-1 self block
            layer(xtab, it, w1t, b1t,
                  lambda t: ddr[:, t * P:(t + 1) * P],
                  dd2, v_dest, lambda t: (xown, t * P),
                  slot_scale=dst_, post_tile=fire_ag)
            # layer 2 -> out_sh (degree-sorted row order; host un-permutes)
            layer(vfull, it2, w2t, b2t,
                  lambda t: ones_row[:, :],
                  dd1, lambda t: (out_sh, t * P), v_dest)

    nc.compile()

    in_maps = []
    for k in range(C):
        in_maps.append({
            "xtab": x_pad, "identp": np.eye(P, dtype=np.float32),
            "xown": x_own_arr[k],
            "dsrc": disrc_arr[k],
            "idxp": idx_arr[k], "idxp2": idx2_arr[k],
            "disd": disd_arr[k], "disd2": disd2_arr[k],
            "disdr": disd_rows_arr[k],
            "w1p": W1, "w2p": W2,
            "b1p": b1.reshape(1, D), "b2p": b2.reshape(1, D),
        })
    global _compiled
    _compiled = (nc, in_maps)
    res = run_bass_kernel_spmd(nc, in_maps, list(range(C)))
    out = np.empty((N, D), dtype=np.float32)
    for k in range(C):
        # rows come back in degree-sorted order; un-permute
        out[k * NSH + perms[k]] = res.results[k]["out_sh"][:NSH]
    return out


def profile_last():
    """Re-run the last compiled program with NTFF tracing; returns exec ns."""
    from concourse.bass_utils import run_bass_kernel_spmd
    assert _compiled is not None
    nc, in_maps = _compiled
    r = run_bass_kernel_spmd(nc, in_maps, list(range(C)), trace=True)
    return r.exec_time_ns

